# revision 33
# baseline (speedup 1.0000x reference)
"""CompressAttn Trainium2 Bass kernel (v2 — pipelined).

Problem: compressed-block attention.
  B=2, N=4096, QH=32, KH=2, D=VD=128, KSZ=32, STRIDE=16, M=255 blocks.
  kc[b,m,h,:] = sum_i w_k[i] * (k[b,16m+i,h,:] + pe_k[i,:])   (same for v)
  out = softmax(q @ kc^T * D^-0.5, causal-banded mask) @ vc, zero for n < 31.

Sharding: 8 cores = (batch b in {0,1}) x (query-head quarter hq in {0..3}).
Each core handles 8 query heads that share a single KV head; K/V compression
runs once per core. No collectives; host gathers.

v2 changes vs v1 (125us):
  - Deep software pipeline: PV+normalize run ~3-4 units behind QK/exp, so
    the PE never stalls waiting on the Scalar exp (each stall resets the
    PE p-state to 1.2GHz for up to 3us -> ~35us of hidden half-clock time).
  - Blocks 0-3 are emitted in PAIRS sharing a [128,1024] psum tile so every
    exp is one 1024-col activation (6 instead of 8 per head).
  - Startup: k load split into small chunks so compression starts as soon
    as the first 96KB lands; v loads moved to the DVE HWDGE queue (the
    gpsimd SWDGE queue burns ~1us of GpSimd engine time per transfer and
    delayed the on-device mask generation); mask variants are generated in
    usage order; the ones-columns of vca are memset on-device.
"""

import ml_dtypes
import numpy as np

import concourse.bacc as bacc
import concourse.mybir as mybir
import concourse.tile as tile
from concourse.bass_utils import run_bass_kernel_spmd

# Problem geometry (hardcoded per contest rules).
B, N, QH, KH, D, VD = 2, 4096, 32, 2, 128, 128
KSZ, STRIDE = 32, 16
M = (N - KSZ) // STRIDE + 1          # 255 compressed blocks (m = 0..254)
HPC = QH // 4                         # 8 query heads per core
NBLK = N // 512                       # 8 query blocks of 512
SM = float(D) ** -0.5
NEGM = -16384.0                       # mask add; exp(SM*(-16384+s)) == 0

F32 = mybir.dt.float32
BF16 = mybir.dt.bfloat16


def build_program():
    nc = bacc.Bacc("TRN2", target_bir_lowering=False, debug=False)

    qT_d = nc.dram_tensor("qT", [HPC, D, N], BF16, kind="ExternalInput")
    # k/v pre-arranged on host to the SBUF tile layout [r, c, d] so loads
    # are one fully-contiguous descriptor per partition
    k_d = nc.dram_tensor("kk", [128, 32 * D], BF16, kind="ExternalInput")
    v_d = nc.dram_tensor("vv", [128, 32 * D], BF16, kind="ExternalInput")
    w01k_d = nc.dram_tensor("w01k", [128, 16], BF16, kind="ExternalInput")
    w01v_d = nc.dram_tensor("w01v", [128, 16], BF16, kind="ExternalInput")
    bk_d = nc.dram_tensor("biask", [128, 1], F32, kind="ExternalInput")
    bv_d = nc.dram_tensor("biasv", [128, 1], F32, kind="ExternalInput")
    o_d = nc.dram_tensor("o", [HPC, 128, N // 128, VD], BF16,
                         kind="ExternalOutput")

    with tile.TileContext(nc) as tc:
        with tc.tile_pool(name="consts", bufs=1) as cp:
            w01k = cp.tile([128, 16], BF16)
            w01v = cp.tile([128, 16], BF16)
            biask = cp.tile([128, 1], F32)
            biasv = cp.tile([128, 1], F32)
            ident = cp.tile([128, 128], F32)
            tmpf = cp.tile([128, 128], F32)
            tmpf2 = cp.tile([128, 128], F32)
            # k/v staged as many small tiles: one dma_start tops out around
            # ~21GB/s, so wide loads must be split to parallelize across
            # the 16 rings
            ka = [cp.tile([128, 4 * 128], BF16, name=f"ka{i}")
                  for i in range(8)]  # k chunks, 4 per tile
            vt = [cp.tile([128, 4 * 128], BF16, name=f"vt{i}")
                  for i in range(8)]  # v chunks, 4 per tile
            kcT = cp.tile([128, 256], BF16)       # [d, m] (col 255 zero pad)
            vcT = cp.tile([128, 256], F32)        # [d, t] staging
            vca0 = cp.tile([128, 130], BF16)      # [m 0:128,   vc|1|0]
            vca1 = cp.tile([128, 130], BF16)      # [m 128:255, vc|1|0]
            q0a0 = cp.tile([128, 1024], BF16)     # head-0 q, blocks 0-1
            q0a1 = cp.tile([128, 1024], BF16)     # head-0 q, blocks 2-3
            q0b = cp.tile([128, 2048], BF16)      # head-0 q, blocks 4-7

            # --- DMA schedule: 3 HWDGE queues in parallel, earliest-needed
            # first; GpSimd (SWDGE) stays free for the mask generation ---
            nc.sync.dma_start(w01k[:, :], w01k_d.ap()[:, :])
            for i in range(5):   # k chunks 0-19 (part A + start of B)
                nc.sync.dma_start(ka[i][:, :],
                                  k_d.ap()[:, 512 * i : 512 * (i + 1)])
            nc.sync.dma_start(q0a0[:, :], qT_d.ap()[0, :, 0:1024])
            nc.sync.dma_start(q0a1[:, :], qT_d.ap()[0, :, 1024:2048])
            nc.scalar.dma_start(biask[:, :], bk_d.ap())
            nc.scalar.dma_start(biasv[:, :], bv_d.ap())
            for i in range(5, 8):   # k chunks 20-31
                nc.scalar.dma_start(ka[i][:, :],
                                    k_d.ap()[:, 512 * i : 512 * (i + 1)])
            for i in range(8):
                eng = nc.scalar if i % 2 else nc.sync
                eng.dma_start(vt[i][:, :],
                              v_d.ap()[:, 512 * i : 512 * (i + 1)])
            nc.scalar.dma_start(q0b[:, :], qT_d.ap()[0, :, 2048:4096])
            nc.gpsimd.dma_start(w01v[:, :], w01v_d.ap()[:, :])

            # --- on-device constants: f32 identity for the V transpose ---
            nc.vector.memset(tmpf[:, :], 1.0)
            nc.gpsimd.affine_select(
                tmpf2[:, :], tmpf[:, :], pattern=[[1, 128]],
                compare_op=mybir.AluOpType.is_ge, fill=0.0,
                base=0, channel_multiplier=-1,
            )
            nc.gpsimd.affine_select(
                ident[:, :], tmpf2[:, :], pattern=[[-1, 128]],
                compare_op=mybir.AluOpType.is_ge, fill=0.0,
                base=0, channel_multiplier=1,
            )
            # vca ones/zero columns (cheaper than a DMA)
            nc.vector.memset(vca0[:, 128:129], 1.0)
            nc.vector.memset(vca0[:, 129:130], 0.0)
            nc.vector.memset(vca1[:, 128:129], 1.0)
            nc.vector.memset(vca1[:, 129:130], 0.0)

            # ---- attention (+ compression woven into head 0) ----
            with (
                tc.tile_pool(name="qp", bufs=5) as qp,
                tc.tile_pool(name="ep", bufs=11) as ep,
                tc.tile_pool(name="op", bufs=5) as op,
                tc.tile_pool(name="rp", bufs=8) as rp,
                tc.tile_pool(name="sps", bufs=2, space="PSUM") as sps,
                tc.tile_pool(name="pvs", bufs=2, space="PSUM") as pvs,
            ):
                pending = []      # fifo of PV work items
                TARGET = 8
                nrm = [0]         # normalize-engine round-robin counter

                def emit_pv_norm(item):
                    b, eT, e0, e1, o_head, h = item
                    # PV psum: col 512*pr + 130*j, tt = 2*pr + j
                    pvt = pvs.tile([128, 1024], F32, tag="pv", name="pvt")
                    for pr in range(2):
                        for j in range(2):
                            tt = 2 * pr + j
                            t = 4 * b + tt
                            K = 8 * t + 7
                            c0k = min(K, 128)
                            c1k = K - 128
                            out_ap = pvt[:, 512 * pr + 130 * j :
                                         512 * pr + 130 * j + 130]
                            nc.tensor.matmul(
                                out_ap,
                                eT[0:c0k, e0 + 128 * tt : e0 + 128 * (tt + 1)],
                                vca0[0:c0k, :],
                                start=True, stop=(c1k <= 0),
                            )
                            if c1k > 0:
                                nc.tensor.matmul(
                                    out_ap,
                                    eT[0:c1k,
                                       e1 + 128 * tt : e1 + 128 * (tt + 1)],
                                    vca1[0:c1k, :],
                                    start=False, stop=True,
                                )
                    # denominators at cols 128 + 512*pr + 130*j
                    pvt3 = pvt[:, :].rearrange("p (pr x) -> p pr x", pr=2)
                    den = pvt3[:, :, 128:259:130]  # [128, 2, 2]
                    rc = rp.tile([128, 4], F32, tag="rc", name="rc")
                    r4 = rc[:, :].rearrange("p (a b) -> p a b", a=2)
                    if b == 0:
                        rtmp = rp.tile([128, 4], F32, tag="rtmp", name="rt")
                        t4 = rtmp[:, :].rearrange("p (a b) -> p a b", a=2)
                        nc.vector.tensor_scalar_add(t4, den, 1e-30)
                        nc.vector.reciprocal(r4, t4)
                    else:
                        nc.vector.reciprocal(r4, den)
                    # one normalization mul: [128, (pr, j, vd)] * rc bcast
                    pv4 = pvt3[:, :, 0:260].rearrange(
                        "p pr (j x) -> p pr j x", j=2
                    )[:, :, :, 0:128]             # [128, 2, 2, 128]
                    rcb = rc[:, :].rearrange(
                        "p (a b) -> p a b", a=2
                    ).unsqueeze(3).broadcast_to([128, 2, 2, 128])
                    dst = o_head[:, 512 * b : 512 * (b + 1)].rearrange(
                        "p (pr j x) -> p pr j x", pr=2, j=2
                    )
                    # alternate the normalize multiply DVE <-> GpSimd: with
                    # the masks on GpSimd, DVE (recip+mul) would otherwise be
                    # the cadence limiter at ~1.1us/unit
                    nrm[0] += 1
                    eng = nc.vector
                    eng.tensor_mul(dst, pv4, rcb)
                    if h == HPC - 1:
                        # last head: store every block so the final exposed
                        # transfer is as small as possible
                        nc.scalar.dma_start(
                            o_d.ap()[h].rearrange("p t v -> p (t v)")[
                                :, 512 * b : 512 * b + 512],
                            o_head[:, 512 * b : 512 * b + 512],
                        )
                    elif b % 2 == 1:
                        qtr = (b - 1) // 2
                        nc.scalar.dma_start(
                            o_d.ap()[h].rearrange("p t v -> p (t v)")[
                                :, 1024 * qtr : 1024 * qtr + 1024],
                            o_head[:, 1024 * qtr : 1024 * qtr + 1024],
                        )

                def drain(n, max_pops=None):
                    # max_pops smooths the DVE recip+multiply load: a burst
                    # of pops saturates DVE and its backlog later stalls
                    # PV via the psum-tile WAW
                    pops = 0
                    while len(pending) > n and (max_pops is None
                                                or pops < max_pops):
                        emit_pv_norm(pending.pop(0))
                        pops += 1

                def emit_mask(eT, col, w, v):
                    # zero eT[p, col+j] where j < 16*p + 31 - 512*v (block
                    # m = chunk_base + p invisible to query), in place on
                    # the idle GpSimd engine instead of PE matmul adds
                    nc.gpsimd.affine_select(
                        eT[:, col : col + w], eT[:, col : col + w],
                        pattern=[[1, w]],
                        compare_op=mybir.AluOpType.is_ge, fill=0.0,
                        base=512 * v - 31, channel_multiplier=-16,
                    )

                def emit_pair(h, be, q_e, q_o, o_head, do_drain=True):
                    # blocks (be, be+1), chunk 0 only, one shared psum tile
                    sT = sps.tile([128, 1024], F32, tag="sT", name="sT")
                    nc.tensor.matmul(sT[:, 0:512], kcT[:, 0:128], q_e,
                                     start=True, stop=True)
                    nc.tensor.matmul(sT[:, 512:1024], kcT[:, 0:128], q_o,
                                     start=True, stop=True)
                    eT = ep.tile([128, 1024], BF16, tag="eT", name="eT")
                    nc.scalar.activation(
                        eT[:, :], sT[:, :],
                        mybir.ActivationFunctionType.Exp, scale=SM,
                    )
                    # adjacent blocks share one affine staircase: the
                    # threshold shifts by exactly the block width, so a
                    # single 1024-col select masks both halves
                    emit_mask(eT, 0, 1024, be)
                    pending.append((be, eT, 0, None, o_head, h))
                    pending.append((be + 1, eT, 512, None, o_head, h))
                    if do_drain:
                        drain(TARGET, max_pops=1)

                def emit_block(h, b, qs, o_head):
                    # block b >= 4: both chunks in one psum tile
                    sT = sps.tile([128, 1024], F32, tag="sT", name="sT")
                    nc.tensor.matmul(sT[:, 0:512], kcT[:, 0:128], qs,
                                     start=True, stop=True)
                    nc.tensor.matmul(sT[:, 512:1024], kcT[:, 128:256], qs,
                                     start=True, stop=True)
                    eT = ep.tile([128, 1024], BF16, tag="eT", name="eT")
                    nc.scalar.activation(
                        eT[:, :], sT[:, :],
                        mybir.ActivationFunctionType.Exp, scale=SM,
                    )
                    if b == 4:
                        # chunk0 staircase clips only rows 126-127, cols<32
                        emit_mask(eT, 0, 32, 4)
                    emit_mask(eT, 512, 512, b - 4)
                    pending.append((b, eT, 0, 512, o_head, h))
                    drain(TARGET, max_pops=2)

                # --- head 0: weave compression into the block pipeline ---
                o_head0 = op.tile([128, N], BF16, tag="o", name="oh")
                # K compression part A (chunks 0-16, psum borrowed from pvs)
                pk = pvs.tile([128, 1024], F32, tag="pv", name="pk")
                for c in range(17):
                    src = ka[c // 4][:, 128 * (c % 4) : 128 * (c % 4 + 1)]
                    nc.tensor.matmul(pk[:, 16 * c : 16 * c + 16], src,
                                     w01k[:, :], start=True, stop=True)
                pk3 = pk[:, 0:512].rearrange("p (t a) -> p t a", a=2)
                # kcT[d,m] = P0[m] + P1[m+1] + bias_k[d] (cols 0:128)
                nc.vector.tensor_scalar_add(kcT[:, 0:128], pk3[:, 0:128, 0],
                                            biask[:, 0:1])
                nc.vector.tensor_add(kcT[:, 0:128], kcT[:, 0:128],
                                     pk3[:, 1:129, 1])
                # QK/exp for blocks 0-3 (no PV yet: vca not ready, so no
                # drain — a drained PV would deadlock the PE queue behind
                # the not-yet-emitted V compression).  Head 1's pairs are
                # woven in too: they only need kcT chunk 0, and they hide
                # the ktB/vtA DMA waits of compression part B / V.
                emit_pair(0, 0, q0a0[:, 0:512], q0a0[:, 512:1024], o_head0,
                          do_drain=False)
                emit_pair(0, 2, q0a1[:, 0:512], q0a1[:, 512:1024], o_head0,
                          do_drain=False)
                # prologue: pairs of head 1 run before compression part B /
                # V to hide the tail of the k/v loads
                qa_t = {}
                qb_t = {}
                oh_t = {0: o_head0}
                qa_t[1] = qp.tile([128, 2048], BF16, tag="qa", name="qa1")
                nc.sync.dma_start(qa_t[1][:, 0:1024], qT_d.ap()[1, :, 0:1024])
                nc.sync.dma_start(qa_t[1][:, 1024:2048],
                                  qT_d.ap()[1, :, 1024:2048])
                for hh in range(1, 2):
                    qa = qa_t[hh]
                    oh_t[hh] = op.tile([128, N], BF16, tag="o", name="ohh")
                    emit_pair(hh, 0, qa[:, 0:512], qa[:, 512:1024],
                              oh_t[hh], do_drain=False)
                    emit_pair(hh, 2, qa[:, 1024:1536], qa[:, 1536:2048],
                              oh_t[hh], do_drain=False)
                    qb_t[hh] = qp.tile([128, 2048], BF16, tag="qb",
                                       name="qbh")
                    nc.gpsimd.dma_start(qb_t[hh][:, :],
                                        qT_d.ap()[hh, :, 2048:4096])
                # K compression part B (chunks 17-31) -> kcT cols 128:255
                for c in range(17, 32):
                    src = ka[c // 4][:, 128 * (c % 4) : 128 * (c % 4 + 1)]
                    nc.tensor.matmul(pk[:, 16 * c : 16 * c + 16], src,
                                     w01k[:, :], start=True, stop=True)
                nc.vector.tensor_scalar_add(kcT[:, 128:M], pk3[:, 128:M, 0],
                                            biask[:, 0:1])
                nc.vector.tensor_add(kcT[:, 128:M], kcT[:, 128:M],
                                     pk3[:, 129 : M + 1, 1])
                nc.vector.memset(kcT[:, M:256], 0.0)
                # V compression
                pv = pvs.tile([128, 1024], F32, tag="pv", name="pvc")
                for c in range(32):
                    src = vt[c // 4][:, 128 * (c % 4) : 128 * (c % 4 + 1)]
                    nc.tensor.matmul(pv[:, 16 * c : 16 * c + 16], src,
                                     w01v[:, :], start=True, stop=True)
                pv3 = pv[:, 0:512].rearrange("p (t a) -> p t a", a=2)
                nc.vector.tensor_scalar_add(vcT[:, 0:M], pv3[:, 0:M, 0],
                                            biasv[:, 0:1])
                nc.vector.tensor_add(vcT[:, 0:M], vcT[:, 0:M],
                                     pv3[:, 1 : M + 1, 1])
                nc.vector.memset(vcT[:, M : M + 1], 0.0)
                tp = pvs.tile([128, 1024], F32, tag="pv", name="tp")
                nc.tensor.transpose(tp[:, 0:128], vcT[:, 0:128], ident[:, :])
                nc.tensor.transpose(tp[:, 128:256], vcT[:, 128:256],
                                    ident[:, :])
                nc.vector.tensor_copy(vca0[:, 0:128], tp[:, 0:128])
                nc.vector.tensor_copy(vca1[:, 0:128], tp[:, 128:256])

                # block phases of heads 0-1, then heads 2-7 in full;
                # q tiles are prefetched one head ahead, in halves so each
                # transfer parallelizes across rings
                def fetch_q(h):
                    qa = qp.tile([128, 2048], BF16, tag="qa", name="qa")
                    qb = qp.tile([128, 2048], BF16, tag="qb", name="qb")
                    nc.sync.dma_start(qa[:, 0:1024], qT_d.ap()[h, :, 0:1024])
                    nc.sync.dma_start(qa[:, 1024:2048],
                                      qT_d.ap()[h, :, 1024:2048])
                    nc.gpsimd.dma_start(qb[:, 0:1024],
                                        qT_d.ap()[h, :, 2048:3072])
                    nc.gpsimd.dma_start(qb[:, 1024:2048],
                                        qT_d.ap()[h, :, 3072:4096])
                    return qa, qb

                qb_t[0] = q0b
                for h in range(2):
                    if h == 1:
                        nextq = fetch_q(2)
                    src = qb_t[h]   # [128, 2048] = blocks 4-7
                    for b in range(4, NBLK):
                        emit_block(h, b,
                                   src[:, 512 * (b - 4) : 512 * (b - 3)],
                                   oh_t[h])
                for h in range(2, HPC):
                    qa, qb = nextq
                    if h + 1 < HPC:
                        nextq = fetch_q(h + 1)
                    o_head = op.tile([128, N], BF16, tag="o", name="oh")
                    emit_pair(h, 0, qa[:, 0:512], qa[:, 512:1024], o_head)
                    emit_pair(h, 2, qa[:, 1024:1536], qa[:, 1536:2048],
                              o_head)
                    for b in range(4, NBLK):
                        emit_block(h, b,
                                   qb[:, 512 * (b - 4) : 512 * (b - 3)],
                                   o_head)
                        if h == HPC - 1:
                            # taper the pipeline so the post-loop tail is
                            # short
                            drain(max(2, TARGET - 2 * (b - 3)))
                drain(0)
    nc.compile()
    return nc


def make_consts(w_k, pe_k, w_v, pe_v):
    """Host-side constant tensors fed to every core."""
    f = np.float32
    w01k = np.zeros((128, 16), f)
    w01v = np.zeros((128, 16), f)
    for r in range(128):
        j = r // 16
        s = r % 16
        for a in range(2):
            # column layout (j, a): col = 2*j + a, matching psum (t, a)
            w01k[r, 2 * j + a] = w_k[16 * a + s]
            w01v[r, 2 * j + a] = w_v[16 * a + s]
    biask = (w_k[:, None] * pe_k).sum(0).astype(f)[:, None]  # [128,1]
    biasv = (w_v[:, None] * pe_v).sum(0).astype(f)[:, None]
    return {
        "w01k": np.ascontiguousarray(w01k).astype(ml_dtypes.bfloat16),
        "w01v": np.ascontiguousarray(w01v).astype(ml_dtypes.bfloat16),
        "biask": np.ascontiguousarray(biask),
        "biasv": np.ascontiguousarray(biasv),
    }


def make_in_map(q, k, v, consts, core):
    b, hq = core // 4, core % 4
    g = hq // 2
    qT = np.ascontiguousarray(
        q[b, :, 8 * hq : 8 * (hq + 1), :].transpose(1, 2, 0)
    ).astype(ml_dtypes.bfloat16)  # [8, D, N]
    # [N, D] -> SBUF tile layout [r=128, c=32, D] (r = row within chunk c)
    kk = np.ascontiguousarray(
        k[b, :, g, :].reshape(32, 128, D).transpose(1, 0, 2).reshape(128, 32 * D)
    ).astype(ml_dtypes.bfloat16)
    vv = np.ascontiguousarray(
        v[b, :, g, :].reshape(32, 128, D).transpose(1, 0, 2).reshape(128, 32 * D)
    ).astype(ml_dtypes.bfloat16)
    return {"qT": qT, "kk": kk, "vv": vv, **consts}


_CACHE = {}


def _compiled():
    if "nc" not in _CACHE:
        _CACHE["nc"] = build_program()
    return _CACHE["nc"]


def kernel(q, k, v, w_k, pe_k, w_v, pe_v, _trace=False, _trace_kwargs=None):
    q = np.asarray(q, np.float32)
    k = np.asarray(k, np.float32)
    v = np.asarray(v, np.float32)
    consts = make_consts(
        np.asarray(w_k, np.float32), np.asarray(pe_k, np.float32),
        np.asarray(w_v, np.float32), np.asarray(pe_v, np.float32),
    )
    nc = _compiled()
    in_maps = [make_in_map(q, k, v, consts, c) for c in range(8)]
    kw = {}
    if _trace:
        kw = {"trace": True, **(_trace_kwargs or {})}
    res = run_bass_kernel_spmd(nc, in_maps, core_ids=list(range(8)), **kw)
    out = np.empty((B, N, QH, VD), np.float32)
    for c in range(8):
        b, hq = c // 4, c % 4
        # o: [HPC, 128 p, 32 t, VD]; query n = 128*t + p
        oc = np.asarray(res.results[c]["o"], dtype=np.float32)
        out[b, :, 8 * hq : 8 * (hq + 1), :] = (
            oc.transpose(2, 1, 0, 3).reshape(N, HPC, VD)
        )
    _CACHE["last_result"] = res
    return out


# revision 34
# speedup vs baseline: 1.1714x; 1.1714x over previous
"""CompressAttn Trainium2 Bass kernel (v2 — pipelined).

Problem: compressed-block attention.
  B=2, N=4096, QH=32, KH=2, D=VD=128, KSZ=32, STRIDE=16, M=255 blocks.
  kc[b,m,h,:] = sum_i w_k[i] * (k[b,16m+i,h,:] + pe_k[i,:])   (same for v)
  out = softmax(q @ kc^T * D^-0.5, causal-banded mask) @ vc, zero for n < 31.

Sharding: 8 cores = (batch b in {0,1}) x (query-head quarter hq in {0..3}).
Each core handles 8 query heads that share a single KV head; K/V compression
runs once per core. No collectives; host gathers.

v2 changes vs v1 (125us):
  - Deep software pipeline: PV+normalize run ~3-4 units behind QK/exp, so
    the PE never stalls waiting on the Scalar exp (each stall resets the
    PE p-state to 1.2GHz for up to 3us -> ~35us of hidden half-clock time).
  - Blocks 0-3 are emitted in PAIRS sharing a [128,1024] psum tile so every
    exp is one 1024-col activation (6 instead of 8 per head).
  - Startup: k load split into small chunks so compression starts as soon
    as the first 96KB lands; v loads moved to the DVE HWDGE queue (the
    gpsimd SWDGE queue burns ~1us of GpSimd engine time per transfer and
    delayed the on-device mask generation); mask variants are generated in
    usage order; the ones-columns of vca are memset on-device.
"""

import ml_dtypes
import numpy as np

import concourse.bacc as bacc
import concourse.mybir as mybir
import concourse.tile as tile
from concourse.bass_utils import run_bass_kernel_spmd

# Problem geometry (hardcoded per contest rules).
B, N, QH, KH, D, VD = 2, 4096, 32, 2, 128, 128
KSZ, STRIDE = 32, 16
M = (N - KSZ) // STRIDE + 1          # 255 compressed blocks (m = 0..254)
HPC = QH // 4                         # 8 query heads per core
NBLK = N // 512                       # 8 query blocks of 512
SM = float(D) ** -0.5
NEGM = -16384.0                       # mask add; exp(SM*(-16384+s)) == 0

F32 = mybir.dt.float32
BF16 = mybir.dt.bfloat16


def build_program():
    nc = bacc.Bacc("TRN2", target_bir_lowering=False, debug=False)

    qT_d = nc.dram_tensor("qT", [HPC, D, N], BF16, kind="ExternalInput")
    # k/v pre-arranged on host to the SBUF tile layout [r, c, d] so loads
    # are one fully-contiguous descriptor per partition
    k_d = nc.dram_tensor("kk", [128, 32 * D], BF16, kind="ExternalInput")
    v_d = nc.dram_tensor("vv", [128, 32 * D], BF16, kind="ExternalInput")
    w01k_d = nc.dram_tensor("w01k", [128, 16], BF16, kind="ExternalInput")
    w01v_d = nc.dram_tensor("w01v", [128, 16], BF16, kind="ExternalInput")
    bk_d = nc.dram_tensor("biask", [128, 1], F32, kind="ExternalInput")
    bv_d = nc.dram_tensor("biasv", [128, 1], F32, kind="ExternalInput")
    o_d = nc.dram_tensor("o", [HPC, 128, N // 128, VD], BF16,
                         kind="ExternalOutput")

    with tile.TileContext(nc) as tc:
        with tc.tile_pool(name="consts", bufs=1) as cp:
            w01k = cp.tile([128, 16], BF16)
            w01v = cp.tile([128, 16], BF16)
            biask = cp.tile([128, 1], F32)
            biasv = cp.tile([128, 1], F32)
            ident = cp.tile([128, 128], F32)
            tmpf = cp.tile([128, 128], F32)
            tmpf2 = cp.tile([128, 128], F32)
            ka = [cp.tile([128, 3 * 128], BF16, name=f"ka{i}")
                  for i in range(3)]  # ch 0-8
            ktA2 = cp.tile([128, 8 * 128], BF16)                    # ch 9-16
            ktB = cp.tile([128, 15 * 128], BF16)                    # ch 17-31
            vtA = cp.tile([128, 16 * 128], BF16)
            vtB = cp.tile([128, 16 * 128], BF16)
            kcT = cp.tile([128, 256], BF16)       # [d, m] (col 255 zero pad)
            vcT = cp.tile([128, 256], F32)        # [d, t] staging
            vca0 = cp.tile([128, 130], BF16)      # [m 0:128,   vc|1|0]
            vca1 = cp.tile([128, 130], BF16)      # [m 128:255, vc|1|0]
            q0a0 = cp.tile([128, 1024], BF16)     # head-0 q, blocks 0-1
            q0a1 = cp.tile([128, 1024], BF16)     # head-0 q, blocks 2-3
            q0b = cp.tile([128, 2048], BF16)      # head-0 q, blocks 4-7

            # --- DMA schedule: 3 HWDGE queues in parallel, earliest-needed
            # first; GpSimd (SWDGE) stays free for the mask generation ---
            nc.sync.dma_start(w01k[:, :], w01k_d.ap()[:, :])
            for i in range(3):
                nc.sync.dma_start(ka[i][:, :],
                                  k_d.ap()[:, 384 * i : 384 * (i + 1)])
            nc.sync.dma_start(ktA2[:, :], k_d.ap()[:, 9 * 128 : 17 * 128])
            nc.sync.dma_start(q0a0[:, :], qT_d.ap()[0, :, 0:1024])
            nc.sync.dma_start(q0a1[:, :], qT_d.ap()[0, :, 1024:2048])
            nc.scalar.dma_start(biask[:, :], bk_d.ap())
            nc.scalar.dma_start(biasv[:, :], bv_d.ap())
            nc.gpsimd.dma_start(w01v[:, :], w01v_d.ap()[:, :])

            # --- on-device constants: f32 identity for the V transpose ---
            nc.vector.memset(tmpf[:, :], 1.0)
            nc.gpsimd.affine_select(
                tmpf2[:, :], tmpf[:, :], pattern=[[1, 128]],
                compare_op=mybir.AluOpType.is_ge, fill=0.0,
                base=0, channel_multiplier=-1,
            )
            nc.gpsimd.affine_select(
                ident[:, :], tmpf2[:, :], pattern=[[-1, 128]],
                compare_op=mybir.AluOpType.is_ge, fill=0.0,
                base=0, channel_multiplier=1,
            )
            # vca ones/zero columns (cheaper than a DMA)
            nc.vector.memset(vca0[:, 128:129], 1.0)
            nc.vector.memset(vca0[:, 129:130], 0.0)
            nc.vector.memset(vca1[:, 128:129], 1.0)
            nc.vector.memset(vca1[:, 129:130], 0.0)

            # ---- attention (+ compression woven into head 0) ----
            with (
                tc.tile_pool(name="qp", bufs=5) as qp,
                tc.tile_pool(name="ep", bufs=11) as ep,
                tc.tile_pool(name="op", bufs=5) as op,
                tc.tile_pool(name="rp", bufs=8) as rp,
                tc.tile_pool(name="sps", bufs=2, space="PSUM") as sps,
                tc.tile_pool(name="pvs", bufs=2, space="PSUM") as pvs,
            ):
                pending = []      # fifo of PV work items
                TARGET = 8
                nrm = [0]         # normalize-engine round-robin counter

                def emit_pv_norm(item):
                    b, eT, e0, e1, o_head, h = item
                    # PV psum: col 512*pr + 130*j, tt = 2*pr + j
                    pvt = pvs.tile([128, 1024], F32, tag="pv", name="pvt")
                    for pr in range(2):
                        for j in range(2):
                            tt = 2 * pr + j
                            t = 4 * b + tt
                            K = 8 * t + 7
                            c0k = min(K, 128)
                            c1k = K - 128
                            out_ap = pvt[:, 512 * pr + 130 * j :
                                         512 * pr + 130 * j + 130]
                            nc.tensor.matmul(
                                out_ap,
                                eT[0:c0k, e0 + 128 * tt : e0 + 128 * (tt + 1)],
                                vca0[0:c0k, :],
                                start=True, stop=(c1k <= 0),
                            )
                            if c1k > 0:
                                nc.tensor.matmul(
                                    out_ap,
                                    eT[0:c1k,
                                       e1 + 128 * tt : e1 + 128 * (tt + 1)],
                                    vca1[0:c1k, :],
                                    start=False, stop=True,
                                )
                    # denominators at cols 128 + 512*pr + 130*j
                    pvt3 = pvt[:, :].rearrange("p (pr x) -> p pr x", pr=2)
                    den = pvt3[:, :, 128:259:130]  # [128, 2, 2]
                    rc = rp.tile([128, 4], F32, tag="rc", name="rc")
                    r4 = rc[:, :].rearrange("p (a b) -> p a b", a=2)
                    if b == 0:
                        rtmp = rp.tile([128, 4], F32, tag="rtmp", name="rt")
                        t4 = rtmp[:, :].rearrange("p (a b) -> p a b", a=2)
                        nc.vector.tensor_scalar_add(t4, den, 1e-30)
                        nc.vector.reciprocal(r4, t4)
                    else:
                        nc.vector.reciprocal(r4, den)
                    # one normalization mul: [128, (pr, j, vd)] * rc bcast
                    pv4 = pvt3[:, :, 0:260].rearrange(
                        "p pr (j x) -> p pr j x", j=2
                    )[:, :, :, 0:128]             # [128, 2, 2, 128]
                    rcb = rc[:, :].rearrange(
                        "p (a b) -> p a b", a=2
                    ).unsqueeze(3).broadcast_to([128, 2, 2, 128])
                    dst = o_head[:, 512 * b : 512 * (b + 1)].rearrange(
                        "p (pr j x) -> p pr j x", pr=2, j=2
                    )
                    # alternate the normalize multiply DVE <-> GpSimd: with
                    # the masks on GpSimd, DVE (recip+mul) would otherwise be
                    # the cadence limiter at ~1.1us/unit
                    nrm[0] += 1
                    eng = nc.vector
                    eng.tensor_mul(dst, pv4, rcb)
                    if h == HPC - 1:
                        # last head: store every block so the final exposed
                        # transfer is as small as possible
                        nc.scalar.dma_start(
                            o_d.ap()[h].rearrange("p t v -> p (t v)")[
                                :, 512 * b : 512 * b + 512],
                            o_head[:, 512 * b : 512 * b + 512],
                        )
                    elif b % 2 == 1:
                        qtr = (b - 1) // 2
                        nc.scalar.dma_start(
                            o_d.ap()[h].rearrange("p t v -> p (t v)")[
                                :, 1024 * qtr : 1024 * qtr + 1024],
                            o_head[:, 1024 * qtr : 1024 * qtr + 1024],
                        )

                def drain(n, max_pops=None):
                    # max_pops smooths the DVE recip+multiply load: a burst
                    # of pops saturates DVE and its backlog later stalls
                    # PV via the psum-tile WAW
                    pops = 0
                    while len(pending) > n and (max_pops is None
                                                or pops < max_pops):
                        emit_pv_norm(pending.pop(0))
                        pops += 1

                def emit_mask(eT, col, w, v):
                    # zero eT[p, col+j] where j < 16*p + 31 - 512*v (block
                    # m = chunk_base + p invisible to query), in place on
                    # the idle GpSimd engine instead of PE matmul adds
                    nc.gpsimd.affine_select(
                        eT[:, col : col + w], eT[:, col : col + w],
                        pattern=[[1, w]],
                        compare_op=mybir.AluOpType.is_ge, fill=0.0,
                        base=512 * v - 31, channel_multiplier=-16,
                    )

                def emit_pair(h, be, q_e, q_o, o_head, do_drain=True):
                    # blocks (be, be+1), chunk 0 only, one shared psum tile
                    sT = sps.tile([128, 1024], F32, tag="sT", name="sT")
                    nc.tensor.matmul(sT[:, 0:512], kcT[:, 0:128], q_e,
                                     start=True, stop=True)
                    nc.tensor.matmul(sT[:, 512:1024], kcT[:, 0:128], q_o,
                                     start=True, stop=True)
                    eT = ep.tile([128, 1024], BF16, tag="eT", name="eT")
                    nc.scalar.activation(
                        eT[:, :], sT[:, :],
                        mybir.ActivationFunctionType.Exp, scale=SM,
                    )
                    # adjacent blocks share one affine staircase: the
                    # threshold shifts by exactly the block width, so a
                    # single 1024-col select masks both halves
                    emit_mask(eT, 0, 1024, be)
                    pending.append((be, eT, 0, None, o_head, h))
                    pending.append((be + 1, eT, 512, None, o_head, h))
                    if do_drain:
                        drain(TARGET)

                def emit_block(h, b, qs, o_head):
                    # block b >= 4: both chunks in one psum tile
                    sT = sps.tile([128, 1024], F32, tag="sT", name="sT")
                    nc.tensor.matmul(sT[:, 0:512], kcT[:, 0:128], qs,
                                     start=True, stop=True)
                    nc.tensor.matmul(sT[:, 512:1024], kcT[:, 128:256], qs,
                                     start=True, stop=True)
                    eT = ep.tile([128, 1024], BF16, tag="eT", name="eT")
                    nc.scalar.activation(
                        eT[:, :], sT[:, :],
                        mybir.ActivationFunctionType.Exp, scale=SM,
                    )
                    if b == 4:
                        # chunk0 staircase clips only rows 126-127, cols<32
                        emit_mask(eT, 0, 32, 4)
                    emit_mask(eT, 512, 512, b - 4)
                    pending.append((b, eT, 0, 512, o_head, h))
                    drain(TARGET)

                # --- head 0: weave compression into the block pipeline ---
                o_head0 = op.tile([128, N], BF16, tag="o", name="oh")
                # K compression part A (chunks 0-16, psum borrowed from pvs)
                pk = pvs.tile([128, 1024], F32, tag="pv", name="pk")
                for c in range(17):
                    src = (ka[c // 3][:, 128 * (c % 3) : 128 * (c % 3 + 1)]
                           if c < 9 else
                           ktA2[:, 128 * (c - 9) : 128 * (c - 8)])
                    nc.tensor.matmul(pk[:, 16 * c : 16 * c + 16], src,
                                     w01k[:, :], start=True, stop=True)
                pk3 = pk[:, 0:512].rearrange("p (t a) -> p t a", a=2)
                # kcT[d,m] = P0[m] + P1[m+1] + bias_k[d] (cols 0:128)
                nc.vector.tensor_scalar_add(kcT[:, 0:128], pk3[:, 0:128, 0],
                                            biask[:, 0:1])
                nc.vector.tensor_add(kcT[:, 0:128], kcT[:, 0:128],
                                     pk3[:, 1:129, 1])
                # QK/exp for blocks 0-3 (no PV yet: vca not ready, so no
                # drain — a drained PV would deadlock the PE queue behind
                # the not-yet-emitted V compression).  Head 1's pairs are
                # woven in too: they only need kcT chunk 0, and they hide
                # the ktB/vtA DMA waits of compression part B / V.
                qa_t = {}
                qb_t = {}
                oh_t = {0: o_head0}
                # qa1 rides the tail of the SP fifo (strictly after k/q0a)
                qa_t[1] = qp.tile([128, 2048], BF16, tag="qa", name="qa1")
                nc.sync.dma_start(qa_t[1][:, :], qT_d.ap()[1, :, 0:2048])
                for hh in range(2, 4):
                    qa_t[hh] = qp.tile([128, 2048], BF16, tag="qa",
                                       name="qah")
                emit_pair(0, 0, q0a0[:, 0:512], q0a0[:, 512:1024], o_head0,
                          do_drain=False)
                # Act-queue-paced loads: each dma_start fires when the
                # preceding exp dispatches (Act engine queue depth is 0),
                # so these transfers never crowd the critical k/q0a stream
                nc.scalar.dma_start(vtA[:, :], v_d.ap()[:, 0 : 16 * 128])
                nc.scalar.dma_start(vtB[:, :], v_d.ap()[:, 16 * 128 :])
                emit_pair(0, 2, q0a1[:, 0:512], q0a1[:, 512:1024], o_head0,
                          do_drain=False)
                nc.scalar.dma_start(qa_t[2][:, :], qT_d.ap()[2, :, 0:2048])
                for hh in range(1, 4):
                    qa = qa_t[hh]
                    oh_t[hh] = op.tile([128, N], BF16, tag="o", name="ohh")
                    emit_pair(hh, 0, qa[:, 0:512], qa[:, 512:1024],
                              oh_t[hh], do_drain=False)
                    if hh == 1:
                        nc.scalar.dma_start(qa_t[3][:, :],
                                            qT_d.ap()[3, :, 0:2048])
                    emit_pair(hh, 2, qa[:, 1024:1536], qa[:, 1536:2048],
                              oh_t[hh], do_drain=False)
                    if hh == 1:
                        nc.scalar.dma_start(ktB[:, :],
                                            k_d.ap()[:, 17 * 128 :])
                        nc.scalar.dma_start(q0b[:, :],
                                            qT_d.ap()[0, :, 2048:4096])
                    qb_t[hh] = qp.tile([128, 2048], BF16, tag="qb",
                                       name="qbh")
                    nc.gpsimd.dma_start(qb_t[hh][:, :],
                                        qT_d.ap()[hh, :, 2048:4096])
                # K compression part B (chunks 17-31) -> kcT cols 128:255
                for c in range(17, 32):
                    src = ktB[:, 128 * (c - 17) : 128 * (c - 16)]
                    nc.tensor.matmul(pk[:, 16 * c : 16 * c + 16], src,
                                     w01k[:, :], start=True, stop=True)
                nc.vector.tensor_scalar_add(kcT[:, 128:M], pk3[:, 128:M, 0],
                                            biask[:, 0:1])
                nc.vector.tensor_add(kcT[:, 128:M], kcT[:, 128:M],
                                     pk3[:, 129 : M + 1, 1])
                nc.vector.memset(kcT[:, M:256], 0.0)
                # V compression
                pv = pvs.tile([128, 1024], F32, tag="pv", name="pvc")
                for c in range(32):
                    src = (vtA[:, 128 * c : 128 * (c + 1)] if c < 16 else
                           vtB[:, 128 * (c - 16) : 128 * (c - 15)])
                    nc.tensor.matmul(pv[:, 16 * c : 16 * c + 16], src,
                                     w01v[:, :], start=True, stop=True)
                pv3 = pv[:, 0:512].rearrange("p (t a) -> p t a", a=2)
                nc.vector.tensor_scalar_add(vcT[:, 0:M], pv3[:, 0:M, 0],
                                            biasv[:, 0:1])
                nc.vector.tensor_add(vcT[:, 0:M], vcT[:, 0:M],
                                     pv3[:, 1 : M + 1, 1])
                nc.vector.memset(vcT[:, M : M + 1], 0.0)
                tp = pvs.tile([128, 1024], F32, tag="pv", name="tp")
                nc.tensor.transpose(tp[:, 0:128], vcT[:, 0:128], ident[:, :])
                nc.tensor.transpose(tp[:, 128:256], vcT[:, 128:256],
                                    ident[:, :])
                nc.vector.tensor_copy(vca0[:, 0:128], tp[:, 0:128])
                nc.vector.tensor_copy(vca1[:, 0:128], tp[:, 128:256])

                def fetch_q(h):
                    qa = qp.tile([128, 2048], BF16, tag="qa", name="qa")
                    qb = qp.tile([128, 2048], BF16, tag="qb", name="qb")
                    nc.sync.dma_start(qa[:, :], qT_d.ap()[h, :, 0:2048])
                    nc.gpsimd.dma_start(qb[:, :], qT_d.ap()[h, :, 2048:4096])
                    return qa, qb

                qb_t[0] = q0b
                for h in range(4):
                    if h == 2:
                        nextq = fetch_q(4)
                    src = qb_t[h]   # [128, 2048] = blocks 4-7
                    for b in range(4, NBLK):
                        emit_block(h, b,
                                   src[:, 512 * (b - 4) : 512 * (b - 3)],
                                   oh_t[h])
                for h in range(4, HPC):
                    qa, qb = nextq
                    if h + 1 < HPC:
                        nextq = fetch_q(h + 1)
                    o_head = op.tile([128, N], BF16, tag="o", name="oh")
                    emit_pair(h, 0, qa[:, 0:512], qa[:, 512:1024], o_head)
                    emit_pair(h, 2, qa[:, 1024:1536], qa[:, 1536:2048],
                              o_head)
                    for b in range(4, NBLK):
                        emit_block(h, b,
                                   qb[:, 512 * (b - 4) : 512 * (b - 3)],
                                   o_head)
                        if h == HPC - 1:
                            # taper the pipeline so the post-loop tail is
                            # short
                            drain(max(2, TARGET - 2 * (b - 3)))
                drain(0)
    nc.compile()
    return nc


def make_consts(w_k, pe_k, w_v, pe_v):
    """Host-side constant tensors fed to every core."""
    f = np.float32
    w01k = np.zeros((128, 16), f)
    w01v = np.zeros((128, 16), f)
    for r in range(128):
        j = r // 16
        s = r % 16
        for a in range(2):
            # column layout (j, a): col = 2*j + a, matching psum (t, a)
            w01k[r, 2 * j + a] = w_k[16 * a + s]
            w01v[r, 2 * j + a] = w_v[16 * a + s]
    biask = (w_k[:, None] * pe_k).sum(0).astype(f)[:, None]  # [128,1]
    biasv = (w_v[:, None] * pe_v).sum(0).astype(f)[:, None]
    return {
        "w01k": np.ascontiguousarray(w01k).astype(ml_dtypes.bfloat16),
        "w01v": np.ascontiguousarray(w01v).astype(ml_dtypes.bfloat16),
        "biask": np.ascontiguousarray(biask),
        "biasv": np.ascontiguousarray(biasv),
    }


def make_in_map(q, k, v, consts, core):
    b, hq = core // 4, core % 4
    g = hq // 2
    qT = np.ascontiguousarray(
        q[b, :, 8 * hq : 8 * (hq + 1), :].transpose(1, 2, 0)
    ).astype(ml_dtypes.bfloat16)  # [8, D, N]
    # [N, D] -> SBUF tile layout [r=128, c=32, D] (r = row within chunk c)
    kk = np.ascontiguousarray(
        k[b, :, g, :].reshape(32, 128, D).transpose(1, 0, 2).reshape(128, 32 * D)
    ).astype(ml_dtypes.bfloat16)
    vv = np.ascontiguousarray(
        v[b, :, g, :].reshape(32, 128, D).transpose(1, 0, 2).reshape(128, 32 * D)
    ).astype(ml_dtypes.bfloat16)
    return {"qT": qT, "kk": kk, "vv": vv, **consts}


_CACHE = {}


def _compiled():
    if "nc" not in _CACHE:
        _CACHE["nc"] = build_program()
    return _CACHE["nc"]


def kernel(q, k, v, w_k, pe_k, w_v, pe_v, _trace=False, _trace_kwargs=None):
    q = np.asarray(q, np.float32)
    k = np.asarray(k, np.float32)
    v = np.asarray(v, np.float32)
    consts = make_consts(
        np.asarray(w_k, np.float32), np.asarray(pe_k, np.float32),
        np.asarray(w_v, np.float32), np.asarray(pe_v, np.float32),
    )
    nc = _compiled()
    in_maps = [make_in_map(q, k, v, consts, c) for c in range(8)]
    kw = {}
    if _trace:
        kw = {"trace": True, **(_trace_kwargs or {})}
    res = run_bass_kernel_spmd(nc, in_maps, core_ids=list(range(8)), **kw)
    out = np.empty((B, N, QH, VD), np.float32)
    for c in range(8):
        b, hq = c // 4, c % 4
        # o: [HPC, 128 p, 32 t, VD]; query n = 128*t + p
        oc = np.asarray(res.results[c]["o"], dtype=np.float32)
        out[b, :, 8 * hq : 8 * (hq + 1), :] = (
            oc.transpose(2, 1, 0, 3).reshape(N, HPC, VD)
        )
    _CACHE["last_result"] = res
    return out


# revision 35
# speedup vs baseline: 1.2159x; 1.0379x over previous
"""CompressAttn Trainium2 Bass kernel (v2 — pipelined).

Problem: compressed-block attention.
  B=2, N=4096, QH=32, KH=2, D=VD=128, KSZ=32, STRIDE=16, M=255 blocks.
  kc[b,m,h,:] = sum_i w_k[i] * (k[b,16m+i,h,:] + pe_k[i,:])   (same for v)
  out = softmax(q @ kc^T * D^-0.5, causal-banded mask) @ vc, zero for n < 31.

Sharding: 8 cores = (batch b in {0,1}) x (query-head quarter hq in {0..3}).
Each core handles 8 query heads that share a single KV head; K/V compression
runs once per core. No collectives; host gathers.

v2 changes vs v1 (125us):
  - Deep software pipeline: PV+normalize run ~3-4 units behind QK/exp, so
    the PE never stalls waiting on the Scalar exp (each stall resets the
    PE p-state to 1.2GHz for up to 3us -> ~35us of hidden half-clock time).
  - Blocks 0-3 are emitted in PAIRS sharing a [128,1024] psum tile so every
    exp is one 1024-col activation (6 instead of 8 per head).
  - Startup: k load split into small chunks so compression starts as soon
    as the first 96KB lands; v loads moved to the DVE HWDGE queue (the
    gpsimd SWDGE queue burns ~1us of GpSimd engine time per transfer and
    delayed the on-device mask generation); mask variants are generated in
    usage order; the ones-columns of vca are memset on-device.
"""

import ml_dtypes
import numpy as np

import concourse.bacc as bacc
import concourse.mybir as mybir
import concourse.tile as tile
from concourse.bass_utils import run_bass_kernel_spmd

# Problem geometry (hardcoded per contest rules).
B, N, QH, KH, D, VD = 2, 4096, 32, 2, 128, 128
KSZ, STRIDE = 32, 16
M = (N - KSZ) // STRIDE + 1          # 255 compressed blocks (m = 0..254)
HPC = QH // 4                         # 8 query heads per core
NBLK = N // 512                       # 8 query blocks of 512
SM = float(D) ** -0.5
NEGM = -16384.0                       # mask add; exp(SM*(-16384+s)) == 0

F32 = mybir.dt.float32
BF16 = mybir.dt.bfloat16


def build_program():
    nc = bacc.Bacc("TRN2", target_bir_lowering=False, debug=False)

    qT_d = nc.dram_tensor("qT", [HPC, D, N], BF16, kind="ExternalInput")
    # k/v pre-arranged on host to the SBUF tile layout [r, c, d] so loads
    # are one fully-contiguous descriptor per partition
    k_d = nc.dram_tensor("kk", [128, 32 * D], BF16, kind="ExternalInput")
    v_d = nc.dram_tensor("vv", [128, 32 * D], BF16, kind="ExternalInput")
    w01k_d = nc.dram_tensor("w01k", [128, 16], BF16, kind="ExternalInput")
    w01v_d = nc.dram_tensor("w01v", [128, 16], BF16, kind="ExternalInput")
    bk_d = nc.dram_tensor("biask", [128, 1], F32, kind="ExternalInput")
    bv_d = nc.dram_tensor("biasv", [128, 1], F32, kind="ExternalInput")
    o_d = nc.dram_tensor("o", [HPC, 128, N // 128, VD], BF16,
                         kind="ExternalOutput")

    with tile.TileContext(nc) as tc:
        with tc.tile_pool(name="consts", bufs=1) as cp:
            w01k = cp.tile([128, 16], BF16)
            w01v = cp.tile([128, 16], BF16)
            biask = cp.tile([128, 1], F32)
            biasv = cp.tile([128, 1], F32)
            ident = cp.tile([128, 128], F32)
            tmpf = cp.tile([128, 128], F32)
            tmpf2 = cp.tile([128, 128], F32)
            ka = [cp.tile([128, 3 * 128], BF16, name=f"ka{i}")
                  for i in range(3)]  # ch 0-8
            ktA2 = cp.tile([128, 8 * 128], BF16)                    # ch 9-16
            ktB = cp.tile([128, 15 * 128], BF16)                    # ch 17-31
            vtA = cp.tile([128, 16 * 128], BF16)
            vtB = cp.tile([128, 16 * 128], BF16)
            kcT = cp.tile([128, 256], BF16)       # [d, m] (col 255 zero pad)
            vcT = cp.tile([128, 256], F32)        # [d, t] staging
            vca0 = cp.tile([128, 130], BF16)      # [m 0:128,   vc|1|0]
            vca1 = cp.tile([128, 130], BF16)      # [m 128:255, vc|1|0]
            q0a0 = cp.tile([128, 1024], BF16)     # head-0 q, blocks 0-1
            q0a1 = cp.tile([128, 1024], BF16)     # head-0 q, blocks 2-3
            q0b = cp.tile([128, 2048], BF16)      # head-0 q, blocks 4-7

            # --- DMA schedule: 3 HWDGE queues in parallel, earliest-needed
            # first; GpSimd (SWDGE) stays free for the mask generation ---
            nc.sync.dma_start(w01k[:, :], w01k_d.ap()[:, :])
            for i in range(3):
                nc.sync.dma_start(ka[i][:, :],
                                  k_d.ap()[:, 384 * i : 384 * (i + 1)])
            nc.sync.dma_start(ktA2[:, :], k_d.ap()[:, 9 * 128 : 17 * 128])
            nc.sync.dma_start(q0a0[:, :], qT_d.ap()[0, :, 0:1024])
            nc.sync.dma_start(q0a1[:, :], qT_d.ap()[0, :, 1024:2048])
            nc.scalar.dma_start(biask[:, :], bk_d.ap())
            nc.scalar.dma_start(biasv[:, :], bv_d.ap())
            nc.gpsimd.dma_start(w01v[:, :], w01v_d.ap()[:, :])

            # --- on-device constants: f32 identity for the V transpose ---
            nc.vector.memset(tmpf[:, :], 1.0)
            nc.gpsimd.affine_select(
                tmpf2[:, :], tmpf[:, :], pattern=[[1, 128]],
                compare_op=mybir.AluOpType.is_ge, fill=0.0,
                base=0, channel_multiplier=-1,
            )
            nc.gpsimd.affine_select(
                ident[:, :], tmpf2[:, :], pattern=[[-1, 128]],
                compare_op=mybir.AluOpType.is_ge, fill=0.0,
                base=0, channel_multiplier=1,
            )
            # vca ones/zero columns (cheaper than a DMA)
            nc.vector.memset(vca0[:, 128:129], 1.0)
            nc.vector.memset(vca0[:, 129:130], 0.0)
            nc.vector.memset(vca1[:, 128:129], 1.0)
            nc.vector.memset(vca1[:, 129:130], 0.0)

            # ---- attention (+ compression woven into head 0) ----
            with (
                tc.tile_pool(name="qp", bufs=5) as qp,
                tc.tile_pool(name="ep", bufs=11) as ep,
                tc.tile_pool(name="op", bufs=5) as op,
                tc.tile_pool(name="rp", bufs=8) as rp,
                tc.tile_pool(name="sps", bufs=2, space="PSUM") as sps,
                tc.tile_pool(name="pvs", bufs=2, space="PSUM") as pvs,
            ):
                pending = []      # fifo of PV work items
                TARGET = 8
                nrm = [0]         # normalize-engine round-robin counter

                def emit_pv_norm(item):
                    b, eT, e0, e1, o_head, h = item
                    # PV psum: col 512*pr + 130*j, tt = 2*pr + j
                    pvt = pvs.tile([128, 1024], F32, tag="pv", name="pvt")
                    for pr in range(2):
                        for j in range(2):
                            tt = 2 * pr + j
                            t = 4 * b + tt
                            K = 8 * t + 7
                            c0k = min(K, 128)
                            c1k = K - 128
                            out_ap = pvt[:, 512 * pr + 130 * j :
                                         512 * pr + 130 * j + 130]
                            nc.tensor.matmul(
                                out_ap,
                                eT[0:c0k, e0 + 128 * tt : e0 + 128 * (tt + 1)],
                                vca0[0:c0k, :],
                                start=True, stop=(c1k <= 0),
                            )
                            if c1k > 0:
                                nc.tensor.matmul(
                                    out_ap,
                                    eT[0:c1k,
                                       e1 + 128 * tt : e1 + 128 * (tt + 1)],
                                    vca1[0:c1k, :],
                                    start=False, stop=True,
                                )
                    # denominators at cols 128 + 512*pr + 130*j
                    pvt3 = pvt[:, :].rearrange("p (pr x) -> p pr x", pr=2)
                    den = pvt3[:, :, 128:259:130]  # [128, 2, 2]
                    rc = rp.tile([128, 4], F32, tag="rc", name="rc")
                    r4 = rc[:, :].rearrange("p (a b) -> p a b", a=2)
                    if b == 0:
                        rtmp = rp.tile([128, 4], F32, tag="rtmp", name="rt")
                        t4 = rtmp[:, :].rearrange("p (a b) -> p a b", a=2)
                        nc.vector.tensor_scalar_add(t4, den, 1e-30)
                        nc.vector.reciprocal(r4, t4)
                    else:
                        nc.vector.reciprocal(r4, den)
                    # one normalization mul: [128, (pr, j, vd)] * rc bcast
                    pv4 = pvt3[:, :, 0:260].rearrange(
                        "p pr (j x) -> p pr j x", j=2
                    )[:, :, :, 0:128]             # [128, 2, 2, 128]
                    rcb = rc[:, :].rearrange(
                        "p (a b) -> p a b", a=2
                    ).unsqueeze(3).broadcast_to([128, 2, 2, 128])
                    dst = o_head[:, 512 * b : 512 * (b + 1)].rearrange(
                        "p (pr j x) -> p pr j x", pr=2, j=2
                    )
                    # alternate the normalize multiply DVE <-> GpSimd: with
                    # the masks on GpSimd, DVE (recip+mul) would otherwise be
                    # the cadence limiter at ~1.1us/unit
                    nrm[0] += 1
                    eng = nc.vector
                    eng.tensor_mul(dst, pv4, rcb)
                    if h == HPC - 1:
                        # last head: store every block so the final exposed
                        # transfer is as small as possible
                        nc.scalar.dma_start(
                            o_d.ap()[h].rearrange("p t v -> p (t v)")[
                                :, 512 * b : 512 * b + 512],
                            o_head[:, 512 * b : 512 * b + 512],
                        )
                    elif b % 2 == 1:
                        qtr = (b - 1) // 2
                        nc.scalar.dma_start(
                            o_d.ap()[h].rearrange("p t v -> p (t v)")[
                                :, 1024 * qtr : 1024 * qtr + 1024],
                            o_head[:, 1024 * qtr : 1024 * qtr + 1024],
                        )

                def drain(n, max_pops=None):
                    # max_pops smooths the DVE recip+multiply load: a burst
                    # of pops saturates DVE and its backlog later stalls
                    # PV via the psum-tile WAW
                    pops = 0
                    while len(pending) > n and (max_pops is None
                                                or pops < max_pops):
                        emit_pv_norm(pending.pop(0))
                        pops += 1

                def emit_mask(eT, col, w, v):
                    # zero eT[p, col+j] where j < 16*p + 31 - 512*v (block
                    # m = chunk_base + p invisible to query), in place on
                    # the idle GpSimd engine instead of PE matmul adds
                    nc.gpsimd.affine_select(
                        eT[:, col : col + w], eT[:, col : col + w],
                        pattern=[[1, w]],
                        compare_op=mybir.AluOpType.is_ge, fill=0.0,
                        base=512 * v - 31, channel_multiplier=-16,
                    )

                def emit_pair(h, be, q_e, q_o, o_head, do_drain=True):
                    # blocks (be, be+1), chunk 0 only, one shared psum tile
                    sT = sps.tile([128, 1024], F32, tag="sT", name="sT")
                    nc.tensor.matmul(sT[:, 0:512], kcT[:, 0:128], q_e,
                                     start=True, stop=True)
                    nc.tensor.matmul(sT[:, 512:1024], kcT[:, 0:128], q_o,
                                     start=True, stop=True)
                    eT = ep.tile([128, 1024], BF16, tag="eT", name="eT")
                    nc.scalar.activation(
                        eT[:, :], sT[:, :],
                        mybir.ActivationFunctionType.Exp, scale=SM,
                    )
                    # adjacent blocks share one affine staircase: the
                    # threshold shifts by exactly the block width, so a
                    # single 1024-col select masks both halves
                    emit_mask(eT, 0, 1024, be)
                    pending.append((be, eT, 0, None, o_head, h))
                    pending.append((be + 1, eT, 512, None, o_head, h))
                    if do_drain:
                        drain(TARGET)

                def emit_block(h, b, qs, o_head):
                    # block b >= 4: both chunks in one psum tile
                    sT = sps.tile([128, 1024], F32, tag="sT", name="sT")
                    nc.tensor.matmul(sT[:, 0:512], kcT[:, 0:128], qs,
                                     start=True, stop=True)
                    nc.tensor.matmul(sT[:, 512:1024], kcT[:, 128:256], qs,
                                     start=True, stop=True)
                    eT = ep.tile([128, 1024], BF16, tag="eT", name="eT")
                    nc.scalar.activation(
                        eT[:, :], sT[:, :],
                        mybir.ActivationFunctionType.Exp, scale=SM,
                    )
                    if b == 4:
                        # chunk0 staircase clips only rows 126-127, cols<32
                        emit_mask(eT, 0, 32, 4)
                    emit_mask(eT, 512, 512, b - 4)
                    pending.append((b, eT, 0, 512, o_head, h))
                    drain(TARGET)

                # --- head 0: weave compression into the block pipeline ---
                o_head0 = op.tile([128, N], BF16, tag="o", name="oh")
                # K compression part A (chunks 0-16, psum borrowed from pvs)
                pk = pvs.tile([128, 1024], F32, tag="pv", name="pk")

                w01kb = w01k[:, 0:1].broadcast_to([128, 512])

                def dummy(n):
                    # junk matmuls (inputs: w01k only, out: unused pk bank1)
                    # to hold the PE p-state at full clock through the
                    # DMA-bound prologue; ~220ns each
                    for _ in range(n):
                        nc.tensor.matmul(pk[0:16, 512:1024], w01k[:, :],
                                         w01kb, start=True, stop=True)

                dummy(10)
                for c in range(17):
                    src = (ka[c // 3][:, 128 * (c % 3) : 128 * (c % 3 + 1)]
                           if c < 9 else
                           ktA2[:, 128 * (c - 9) : 128 * (c - 8)])
                    nc.tensor.matmul(pk[:, 16 * c : 16 * c + 16], src,
                                     w01k[:, :], start=True, stop=True)
                    if c % 3 == 2:
                        dummy(3)
                dummy(12)
                pk3 = pk[:, 0:512].rearrange("p (t a) -> p t a", a=2)
                # kcT[d,m] = P0[m] + P1[m+1] + bias_k[d] (cols 0:128)
                nc.vector.tensor_scalar_add(kcT[:, 0:128], pk3[:, 0:128, 0],
                                            biask[:, 0:1])
                nc.vector.tensor_add(kcT[:, 0:128], kcT[:, 0:128],
                                     pk3[:, 1:129, 1])
                # QK/exp for blocks 0-3 (no PV yet: vca not ready, so no
                # drain — a drained PV would deadlock the PE queue behind
                # the not-yet-emitted V compression).  Head 1's pairs are
                # woven in too: they only need kcT chunk 0, and they hide
                # the ktB/vtA DMA waits of compression part B / V.
                qa_t = {}
                qb_t = {}
                oh_t = {0: o_head0}
                # qa1 rides the tail of the SP fifo (strictly after k/q0a)
                qa_t[1] = qp.tile([128, 2048], BF16, tag="qa", name="qa1")
                nc.sync.dma_start(qa_t[1][:, :], qT_d.ap()[1, :, 0:2048])
                for hh in range(2, 4):
                    qa_t[hh] = qp.tile([128, 2048], BF16, tag="qa",
                                       name="qah")
                dummy(6)
                emit_pair(0, 0, q0a0[:, 0:512], q0a0[:, 512:1024], o_head0,
                          do_drain=False)
                dummy(4)
                # Act-queue-paced loads: each dma_start fires when the
                # preceding exp dispatches (Act engine queue depth is 0),
                # so these transfers never crowd the critical k/q0a stream
                nc.scalar.dma_start(vtA[:, :], v_d.ap()[:, 0 : 16 * 128])
                nc.scalar.dma_start(vtB[:, :], v_d.ap()[:, 16 * 128 :])
                emit_pair(0, 2, q0a1[:, 0:512], q0a1[:, 512:1024], o_head0,
                          do_drain=False)
                dummy(4)
                nc.scalar.dma_start(qa_t[2][:, :], qT_d.ap()[2, :, 0:2048])
                for hh in range(1, 4):
                    qa = qa_t[hh]
                    oh_t[hh] = op.tile([128, N], BF16, tag="o", name="ohh")
                    emit_pair(hh, 0, qa[:, 0:512], qa[:, 512:1024],
                              oh_t[hh], do_drain=False)
                    dummy(4)
                    if hh == 1:
                        nc.scalar.dma_start(qa_t[3][:, :],
                                            qT_d.ap()[3, :, 0:2048])
                    emit_pair(hh, 2, qa[:, 1024:1536], qa[:, 1536:2048],
                              oh_t[hh], do_drain=False)
                    if hh == 1:
                        nc.scalar.dma_start(ktB[:, :],
                                            k_d.ap()[:, 17 * 128 :])
                        nc.scalar.dma_start(q0b[:, :],
                                            qT_d.ap()[0, :, 2048:4096])
                    qb_t[hh] = qp.tile([128, 2048], BF16, tag="qb",
                                       name="qbh")
                    nc.scalar.dma_start(qb_t[hh][:, :],
                                        qT_d.ap()[hh, :, 2048:4096])
                # K compression part B (chunks 17-31) -> kcT cols 128:255
                for c in range(17, 32):
                    src = ktB[:, 128 * (c - 17) : 128 * (c - 16)]
                    nc.tensor.matmul(pk[:, 16 * c : 16 * c + 16], src,
                                     w01k[:, :], start=True, stop=True)
                nc.vector.tensor_scalar_add(kcT[:, 128:M], pk3[:, 128:M, 0],
                                            biask[:, 0:1])
                nc.vector.tensor_add(kcT[:, 128:M], kcT[:, 128:M],
                                     pk3[:, 129 : M + 1, 1])
                nc.vector.memset(kcT[:, M:256], 0.0)
                # V compression
                pv = pvs.tile([128, 1024], F32, tag="pv", name="pvc")
                for c in range(32):
                    src = (vtA[:, 128 * c : 128 * (c + 1)] if c < 16 else
                           vtB[:, 128 * (c - 16) : 128 * (c - 15)])
                    nc.tensor.matmul(pv[:, 16 * c : 16 * c + 16], src,
                                     w01v[:, :], start=True, stop=True)
                pv3 = pv[:, 0:512].rearrange("p (t a) -> p t a", a=2)
                nc.vector.tensor_scalar_add(vcT[:, 0:M], pv3[:, 0:M, 0],
                                            biasv[:, 0:1])
                nc.vector.tensor_add(vcT[:, 0:M], vcT[:, 0:M],
                                     pv3[:, 1 : M + 1, 1])
                nc.vector.memset(vcT[:, M : M + 1], 0.0)
                tp = pvs.tile([128, 1024], F32, tag="pv", name="tp")
                nc.tensor.transpose(tp[:, 0:128], vcT[:, 0:128], ident[:, :])
                nc.tensor.transpose(tp[:, 128:256], vcT[:, 128:256],
                                    ident[:, :])
                nc.vector.tensor_copy(vca0[:, 0:128], tp[:, 0:128])
                nc.vector.tensor_copy(vca1[:, 0:128], tp[:, 128:256])

                def fetch_q(h):
                    qa = qp.tile([128, 2048], BF16, tag="qa", name="qa")
                    qb = qp.tile([128, 2048], BF16, tag="qb", name="qb")
                    nc.sync.dma_start(qa[:, :], qT_d.ap()[h, :, 0:2048])
                    nc.scalar.dma_start(qb[:, :], qT_d.ap()[h, :, 2048:4096])
                    return qa, qb

                qb_t[0] = q0b
                for h in range(4):
                    if h == 2:
                        nextq = fetch_q(4)
                    src = qb_t[h]   # [128, 2048] = blocks 4-7
                    for b in range(4, NBLK):
                        emit_block(h, b,
                                   src[:, 512 * (b - 4) : 512 * (b - 3)],
                                   oh_t[h])
                for h in range(4, HPC):
                    qa, qb = nextq
                    if h + 1 < HPC:
                        nextq = fetch_q(h + 1)
                    o_head = op.tile([128, N], BF16, tag="o", name="oh")
                    emit_pair(h, 0, qa[:, 0:512], qa[:, 512:1024], o_head)
                    emit_pair(h, 2, qa[:, 1024:1536], qa[:, 1536:2048],
                              o_head)
                    for b in range(4, NBLK):
                        emit_block(h, b,
                                   qb[:, 512 * (b - 4) : 512 * (b - 3)],
                                   o_head)
                        if h == HPC - 1:
                            # taper the pipeline so the post-loop tail is
                            # short
                            drain(max(2, TARGET - 2 * (b - 3)))
                drain(0)
    nc.compile()
    return nc


def make_consts(w_k, pe_k, w_v, pe_v):
    """Host-side constant tensors fed to every core."""
    f = np.float32
    w01k = np.zeros((128, 16), f)
    w01v = np.zeros((128, 16), f)
    for r in range(128):
        j = r // 16
        s = r % 16
        for a in range(2):
            # column layout (j, a): col = 2*j + a, matching psum (t, a)
            w01k[r, 2 * j + a] = w_k[16 * a + s]
            w01v[r, 2 * j + a] = w_v[16 * a + s]
    biask = (w_k[:, None] * pe_k).sum(0).astype(f)[:, None]  # [128,1]
    biasv = (w_v[:, None] * pe_v).sum(0).astype(f)[:, None]
    return {
        "w01k": np.ascontiguousarray(w01k).astype(ml_dtypes.bfloat16),
        "w01v": np.ascontiguousarray(w01v).astype(ml_dtypes.bfloat16),
        "biask": np.ascontiguousarray(biask),
        "biasv": np.ascontiguousarray(biasv),
    }


def make_in_map(q, k, v, consts, core):
    b, hq = core // 4, core % 4
    g = hq // 2
    qT = np.ascontiguousarray(
        q[b, :, 8 * hq : 8 * (hq + 1), :].transpose(1, 2, 0)
    ).astype(ml_dtypes.bfloat16)  # [8, D, N]
    # [N, D] -> SBUF tile layout [r=128, c=32, D] (r = row within chunk c)
    kk = np.ascontiguousarray(
        k[b, :, g, :].reshape(32, 128, D).transpose(1, 0, 2).reshape(128, 32 * D)
    ).astype(ml_dtypes.bfloat16)
    vv = np.ascontiguousarray(
        v[b, :, g, :].reshape(32, 128, D).transpose(1, 0, 2).reshape(128, 32 * D)
    ).astype(ml_dtypes.bfloat16)
    return {"qT": qT, "kk": kk, "vv": vv, **consts}


_CACHE = {}


def _compiled():
    if "nc" not in _CACHE:
        _CACHE["nc"] = build_program()
    return _CACHE["nc"]


def kernel(q, k, v, w_k, pe_k, w_v, pe_v, _trace=False, _trace_kwargs=None):
    q = np.asarray(q, np.float32)
    k = np.asarray(k, np.float32)
    v = np.asarray(v, np.float32)
    consts = make_consts(
        np.asarray(w_k, np.float32), np.asarray(pe_k, np.float32),
        np.asarray(w_v, np.float32), np.asarray(pe_v, np.float32),
    )
    nc = _compiled()
    in_maps = [make_in_map(q, k, v, consts, c) for c in range(8)]
    kw = {}
    if _trace:
        kw = {"trace": True, **(_trace_kwargs or {})}
    res = run_bass_kernel_spmd(nc, in_maps, core_ids=list(range(8)), **kw)
    out = np.empty((B, N, QH, VD), np.float32)
    for c in range(8):
        b, hq = c // 4, c % 4
        # o: [HPC, 128 p, 32 t, VD]; query n = 128*t + p
        oc = np.asarray(res.results[c]["o"], dtype=np.float32)
        out[b, :, 8 * hq : 8 * (hq + 1), :] = (
            oc.transpose(2, 1, 0, 3).reshape(N, HPC, VD)
        )
    _CACHE["last_result"] = res
    return out


# revision 36
# speedup vs baseline: 1.2949x; 1.0650x over previous
"""CompressAttn Trainium2 Bass kernel (v10).

Problem: compressed-block attention.
  B=2, N=4096, QH=32, KH=2, D=VD=128, KSZ=32, STRIDE=16, M=255 blocks.
  kc[b,m,h,:] = sum_i w_k[i] * (k[b,16m+i,h,:] + pe_k[i,:])   (same for v)
  out = softmax(q @ kc^T * D^-0.5, causal-banded mask) @ vc, zero for n < 31.

Sharding: 8 cores = (batch b in {0,1}) x (query-head quarter hq in {0..3}).
Each core handles 8 query heads that share a single KV head; K/V compression
runs once per core. No collectives; host gathers (and zeroes the n<31 rows).

Device structure (evolution from the 125us v1):
  - Scores in [m, q] psum tiles; blocks 0-3 emitted as PAIRS sharing one
    [128,1024] tile so every exp is a single 1024-col activation (48 total,
    the Scalar-engine floor).
  - The causal staircase mask moved off the PE entirely: it is an in-place
    GpSimd affine_select on eT after the exp (fill=0).  One select covers a
    whole pair, because the threshold shifts by exactly the block width
    between adjacent blocks (same affine form).
  - Deep software pipeline: PV+normalize run up to 8 items behind QK/exp
    (TARGET), so PV's stationary (eT) is always long-ready and the PE
    p-state (1.2GHz until 3us of continuous busy) stays at 2.4GHz.
  - PV keeps the denominator embedded (ones column in the vca moving
    operand); psum bank geometry (4 x 130 > 512) pins pvt tiles to 2 banks
    x 2 bufs, which makes the DVE recip+multiply the psum-recycle path.
  - Startup: one dma_start moves ~21GB/s, transfers on one HWDGE fifo are
    strictly ordered, and un-paced triggers all fire at t~7us.  So the SP
    fifo carries only the critical chain (w01k, k part A chunks, ktA2,
    q0a, qa1) and everything else (v, ktB, q0b, qa2/3, qb*) fires from
    Act-queue slots between specific exps -- the Act sequencer has engine
    queue depth 0, so those triggers are naturally paced by the pipeline.
  - Junk 512-col matmuls (into a dead psum region, inputs = the tiny w01k
    tile) fill the DMA-bound prologue so the PE p-state ramp never resets
    before cruise.
  - Pairs of heads 1-3 run before compression part B / V compression to
    hide the k/v load tail; block phases follow, then heads 4-7 in full.

Timing: ~116-122us HW exec (device DVFS gives +-5us run-to-run) vs 125.6us
for v1 as officially measured / 151.7us re-measured this session.
"""
import ml_dtypes
import numpy as np

import concourse.bacc as bacc
import concourse.mybir as mybir
import concourse.tile as tile
from concourse.bass_utils import run_bass_kernel_spmd

# Problem geometry (hardcoded per contest rules).
B, N, QH, KH, D, VD = 2, 4096, 32, 2, 128, 128
KSZ, STRIDE = 32, 16
M = (N - KSZ) // STRIDE + 1          # 255 compressed blocks (m = 0..254)
HPC = QH // 4                         # 8 query heads per core
NBLK = N // 512                       # 8 query blocks of 512
SM = float(D) ** -0.5
NEGM = -16384.0                       # mask add; exp(SM*(-16384+s)) == 0

F32 = mybir.dt.float32
BF16 = mybir.dt.bfloat16


def build_program():
    nc = bacc.Bacc("TRN2", target_bir_lowering=False, debug=False)

    qT_d = nc.dram_tensor("qT", [HPC, D, N], BF16, kind="ExternalInput")
    # k/v pre-arranged on host to the SBUF tile layout [r, c, d] so loads
    # are one fully-contiguous descriptor per partition
    k_d = nc.dram_tensor("kk", [128, 32 * D], BF16, kind="ExternalInput")
    v_d = nc.dram_tensor("vv", [128, 32 * D], BF16, kind="ExternalInput")
    w01k_d = nc.dram_tensor("w01k", [128, 16], BF16, kind="ExternalInput")
    w01v_d = nc.dram_tensor("w01v", [128, 16], BF16, kind="ExternalInput")
    bk_d = nc.dram_tensor("biask", [128, 1], F32, kind="ExternalInput")
    bv_d = nc.dram_tensor("biasv", [128, 1], F32, kind="ExternalInput")
    o_d = nc.dram_tensor("o", [HPC, 128, N // 128, VD], BF16,
                         kind="ExternalOutput")

    with tile.TileContext(nc) as tc:
        with tc.tile_pool(name="consts", bufs=1) as cp:
            w01k = cp.tile([128, 16], BF16)
            w01v = cp.tile([128, 16], BF16)
            biask = cp.tile([128, 1], F32)
            biasv = cp.tile([128, 1], F32)
            ident = cp.tile([128, 128], F32)
            tmpf = cp.tile([128, 128], F32)
            tmpf2 = cp.tile([128, 128], F32)
            ka = [cp.tile([128, 3 * 128], BF16, name=f"ka{i}")
                  for i in range(3)]  # ch 0-8
            ktA2 = cp.tile([128, 8 * 128], BF16)                    # ch 9-16
            ktB = cp.tile([128, 15 * 128], BF16)                    # ch 17-31
            vtA = cp.tile([128, 16 * 128], BF16)
            vtB = cp.tile([128, 16 * 128], BF16)
            kcT = cp.tile([128, 256], BF16)       # [d, m] (col 255 zero pad)
            vcT = cp.tile([128, 256], F32)        # [d, t] staging
            vca0 = cp.tile([128, 130], BF16)      # [m 0:128,   vc|1|0]
            vca1 = cp.tile([128, 130], BF16)      # [m 128:255, vc|1|0]
            q0a0 = cp.tile([128, 1024], BF16)     # head-0 q, blocks 0-1
            q0a1 = cp.tile([128, 1024], BF16)     # head-0 q, blocks 2-3
            q0b = cp.tile([128, 2048], BF16)      # head-0 q, blocks 4-7

            # --- DMA schedule: 3 HWDGE queues in parallel, earliest-needed
            # first; GpSimd (SWDGE) stays free for the mask generation ---
            nc.sync.dma_start(w01k[:, :], w01k_d.ap()[:, :])
            for i in range(3):
                nc.sync.dma_start(ka[i][:, :],
                                  k_d.ap()[:, 384 * i : 384 * (i + 1)])
            nc.sync.dma_start(ktA2[:, :], k_d.ap()[:, 9 * 128 : 17 * 128])
            nc.sync.dma_start(q0a0[:, :], qT_d.ap()[0, :, 0:1024])
            nc.sync.dma_start(q0a1[:, :], qT_d.ap()[0, :, 1024:2048])
            nc.scalar.dma_start(biask[:, :], bk_d.ap())
            nc.scalar.dma_start(biasv[:, :], bv_d.ap())
            nc.gpsimd.dma_start(w01v[:, :], w01v_d.ap()[:, :])

            # --- on-device constants: f32 identity for the V transpose ---
            nc.vector.memset(tmpf[:, :], 1.0)
            nc.gpsimd.affine_select(
                tmpf2[:, :], tmpf[:, :], pattern=[[1, 128]],
                compare_op=mybir.AluOpType.is_ge, fill=0.0,
                base=0, channel_multiplier=-1,
            )
            nc.gpsimd.affine_select(
                ident[:, :], tmpf2[:, :], pattern=[[-1, 128]],
                compare_op=mybir.AluOpType.is_ge, fill=0.0,
                base=0, channel_multiplier=1,
            )
            # vca ones/zero columns (cheaper than a DMA)
            nc.vector.memset(vca0[:, 128:129], 1.0)
            nc.vector.memset(vca0[:, 129:130], 0.0)
            nc.vector.memset(vca1[:, 128:129], 1.0)
            nc.vector.memset(vca1[:, 129:130], 0.0)

            # ---- attention (+ compression woven into head 0) ----
            with (
                tc.tile_pool(name="qp", bufs=5) as qp,
                tc.tile_pool(name="ep", bufs=11) as ep,
                tc.tile_pool(name="op", bufs=5) as op,
                tc.tile_pool(name="rp", bufs=8) as rp,
                tc.tile_pool(name="sps", bufs=2, space="PSUM") as sps,
                tc.tile_pool(name="pvs", bufs=2, space="PSUM") as pvs,
            ):
                pending = []      # fifo of PV work items
                TARGET = 8
                nrm = [0]         # normalize-engine round-robin counter

                def emit_pv_norm(item):
                    b, eT, e0, e1, o_head, h = item
                    # PV psum: col 512*pr + 130*j, tt = 2*pr + j
                    pvt = pvs.tile([128, 1024], F32, tag="pv", name="pvt")
                    for pr in range(2):
                        for j in range(2):
                            tt = 2 * pr + j
                            t = 4 * b + tt
                            K = 8 * t + 7
                            c0k = min(K, 128)
                            c1k = K - 128
                            out_ap = pvt[:, 512 * pr + 130 * j :
                                         512 * pr + 130 * j + 130]
                            nc.tensor.matmul(
                                out_ap,
                                eT[0:c0k, e0 + 128 * tt : e0 + 128 * (tt + 1)],
                                vca0[0:c0k, :],
                                start=True, stop=(c1k <= 0),
                            )
                            if c1k > 0:
                                nc.tensor.matmul(
                                    out_ap,
                                    eT[0:c1k,
                                       e1 + 128 * tt : e1 + 128 * (tt + 1)],
                                    vca1[0:c1k, :],
                                    start=False, stop=True,
                                )
                    # denominators at cols 128 + 512*pr + 130*j
                    pvt3 = pvt[:, :].rearrange("p (pr x) -> p pr x", pr=2)
                    den = pvt3[:, :, 128:259:130]  # [128, 2, 2]
                    rc = rp.tile([128, 4], F32, tag="rc", name="rc")
                    r4 = rc[:, :].rearrange("p (a b) -> p a b", a=2)
                    # block 0 rows n<31 divide 0/0 -> NaN; the host gather
                    # overwrites those rows with the exact zeros
                    nc.vector.reciprocal(r4, den)
                    # one normalization mul: [128, (pr, j, vd)] * rc bcast
                    pv4 = pvt3[:, :, 0:260].rearrange(
                        "p pr (j x) -> p pr j x", j=2
                    )[:, :, :, 0:128]             # [128, 2, 2, 128]
                    rcb = rc[:, :].rearrange(
                        "p (a b) -> p a b", a=2
                    ).unsqueeze(3).broadcast_to([128, 2, 2, 128])
                    dst = o_head[:, 512 * b : 512 * (b + 1)].rearrange(
                        "p (pr j x) -> p pr j x", pr=2, j=2
                    )
                    # alternate the normalize multiply DVE <-> GpSimd: with
                    # the masks on GpSimd, DVE (recip+mul) would otherwise be
                    # the cadence limiter at ~1.1us/unit
                    nrm[0] += 1
                    eng = nc.vector
                    eng.tensor_mul(dst, pv4, rcb)
                    if h == HPC - 1:
                        # last head: store every block so the final exposed
                        # transfer is as small as possible
                        nc.scalar.dma_start(
                            o_d.ap()[h].rearrange("p t v -> p (t v)")[
                                :, 512 * b : 512 * b + 512],
                            o_head[:, 512 * b : 512 * b + 512],
                        )
                    elif b % 2 == 1:
                        qtr = (b - 1) // 2
                        nc.scalar.dma_start(
                            o_d.ap()[h].rearrange("p t v -> p (t v)")[
                                :, 1024 * qtr : 1024 * qtr + 1024],
                            o_head[:, 1024 * qtr : 1024 * qtr + 1024],
                        )

                def drain(n, max_pops=None):
                    # max_pops smooths the DVE recip+multiply load: a burst
                    # of pops saturates DVE and its backlog later stalls
                    # PV via the psum-tile WAW
                    pops = 0
                    while len(pending) > n and (max_pops is None
                                                or pops < max_pops):
                        emit_pv_norm(pending.pop(0))
                        pops += 1

                def emit_mask(eT, col, w, v):
                    # zero eT[p, col+j] where j < 16*p + 31 - 512*v (block
                    # m = chunk_base + p invisible to query), in place on
                    # the idle GpSimd engine instead of PE matmul adds
                    nc.gpsimd.affine_select(
                        eT[:, col : col + w], eT[:, col : col + w],
                        pattern=[[1, w]],
                        compare_op=mybir.AluOpType.is_ge, fill=0.0,
                        base=512 * v - 31, channel_multiplier=-16,
                    )

                def emit_pair(h, be, q_e, q_o, o_head, do_drain=True):
                    # blocks (be, be+1), chunk 0 only, one shared psum tile
                    sT = sps.tile([128, 1024], F32, tag="sT", name="sT")
                    nc.tensor.matmul(sT[:, 0:512], kcT[:, 0:128], q_e,
                                     start=True, stop=True)
                    nc.tensor.matmul(sT[:, 512:1024], kcT[:, 0:128], q_o,
                                     start=True, stop=True)
                    eT = ep.tile([128, 1024], BF16, tag="eT", name="eT")
                    nc.scalar.activation(
                        eT[:, :], sT[:, :],
                        mybir.ActivationFunctionType.Exp, scale=SM,
                    )
                    # adjacent blocks share one affine staircase: the
                    # threshold shifts by exactly the block width, so a
                    # single 1024-col select masks both halves
                    emit_mask(eT, 0, 1024, be)
                    pending.append((be, eT, 0, None, o_head, h))
                    pending.append((be + 1, eT, 512, None, o_head, h))
                    if do_drain:
                        drain(TARGET)

                def emit_block(h, b, qs, o_head):
                    # block b >= 4: both chunks in one psum tile
                    sT = sps.tile([128, 1024], F32, tag="sT", name="sT")
                    nc.tensor.matmul(sT[:, 0:512], kcT[:, 0:128], qs,
                                     start=True, stop=True)
                    nc.tensor.matmul(sT[:, 512:1024], kcT[:, 128:256], qs,
                                     start=True, stop=True)
                    eT = ep.tile([128, 1024], BF16, tag="eT", name="eT")
                    nc.scalar.activation(
                        eT[:, :], sT[:, :],
                        mybir.ActivationFunctionType.Exp, scale=SM,
                    )
                    if b == 4:
                        # chunk0 staircase clips only rows 126-127, cols<32
                        emit_mask(eT, 0, 32, 4)
                    emit_mask(eT, 512, 512, b - 4)
                    pending.append((b, eT, 0, 512, o_head, h))
                    drain(TARGET)

                # --- head 0: weave compression into the block pipeline ---
                o_head0 = op.tile([128, N], BF16, tag="o", name="oh")
                # K compression part A (chunks 0-16, psum borrowed from pvs)
                pk = pvs.tile([128, 1024], F32, tag="pv", name="pk")

                w01kb = w01k[:, 0:1].broadcast_to([128, 512])

                def dummy(n):
                    # junk matmuls (inputs: w01k only, out: unused pk bank1)
                    # to hold the PE p-state at full clock through the
                    # DMA-bound prologue; ~220ns each
                    for _ in range(n):
                        nc.tensor.matmul(pk[0:16, 512:1024], w01k[:, :],
                                         w01kb, start=True, stop=True)

                dummy(10)
                for c in range(17):
                    src = (ka[c // 3][:, 128 * (c % 3) : 128 * (c % 3 + 1)]
                           if c < 9 else
                           ktA2[:, 128 * (c - 9) : 128 * (c - 8)])
                    nc.tensor.matmul(pk[:, 16 * c : 16 * c + 16], src,
                                     w01k[:, :], start=True, stop=True)
                    if c % 3 == 2:
                        dummy(3)
                dummy(12)
                pk3 = pk[:, 0:512].rearrange("p (t a) -> p t a", a=2)
                # kcT[d,m] = P0[m] + P1[m+1] + bias_k[d] (cols 0:128)
                nc.vector.tensor_scalar_add(kcT[:, 0:128], pk3[:, 0:128, 0],
                                            biask[:, 0:1])
                nc.vector.tensor_add(kcT[:, 0:128], kcT[:, 0:128],
                                     pk3[:, 1:129, 1])
                # QK/exp for blocks 0-3 (no PV yet: vca not ready, so no
                # drain — a drained PV would deadlock the PE queue behind
                # the not-yet-emitted V compression).  Head 1's pairs are
                # woven in too: they only need kcT chunk 0, and they hide
                # the ktB/vtA DMA waits of compression part B / V.
                qa_t = {}
                qb_t = {}
                oh_t = {0: o_head0}
                # qa1 rides the tail of the SP fifo (strictly after k/q0a)
                qa_t[1] = qp.tile([128, 2048], BF16, tag="qa", name="qa1")
                nc.sync.dma_start(qa_t[1][:, :], qT_d.ap()[1, :, 0:2048])
                for hh in range(2, 4):
                    qa_t[hh] = qp.tile([128, 2048], BF16, tag="qa",
                                       name="qah")
                dummy(6)
                emit_pair(0, 0, q0a0[:, 0:512], q0a0[:, 512:1024], o_head0,
                          do_drain=False)
                dummy(4)
                # Act-queue-paced loads: each dma_start fires when the
                # preceding exp dispatches (Act engine queue depth is 0),
                # so these transfers never crowd the critical k/q0a stream
                nc.scalar.dma_start(vtA[:, :], v_d.ap()[:, 0 : 16 * 128])
                nc.scalar.dma_start(vtB[:, :], v_d.ap()[:, 16 * 128 :])
                emit_pair(0, 2, q0a1[:, 0:512], q0a1[:, 512:1024], o_head0,
                          do_drain=False)
                dummy(4)
                nc.scalar.dma_start(qa_t[2][:, :], qT_d.ap()[2, :, 0:2048])
                for hh in range(1, 4):
                    qa = qa_t[hh]
                    oh_t[hh] = op.tile([128, N], BF16, tag="o", name="ohh")
                    emit_pair(hh, 0, qa[:, 0:512], qa[:, 512:1024],
                              oh_t[hh], do_drain=False)
                    dummy(4)
                    if hh == 1:
                        nc.scalar.dma_start(qa_t[3][:, :],
                                            qT_d.ap()[3, :, 0:2048])
                        nc.scalar.dma_start(ktB[:, :],
                                            k_d.ap()[:, 17 * 128 :])
                        nc.scalar.dma_start(q0b[:, :],
                                            qT_d.ap()[0, :, 2048:4096])
                    emit_pair(hh, 2, qa[:, 1024:1536], qa[:, 1536:2048],
                              oh_t[hh], do_drain=False)
                    qb_t[hh] = qp.tile([128, 2048], BF16, tag="qb",
                                       name="qbh")
                    nc.scalar.dma_start(qb_t[hh][:, :],
                                        qT_d.ap()[hh, :, 2048:4096])
                # K compression part B (chunks 17-31) -> kcT cols 128:255
                for c in range(17, 32):
                    src = ktB[:, 128 * (c - 17) : 128 * (c - 16)]
                    nc.tensor.matmul(pk[:, 16 * c : 16 * c + 16], src,
                                     w01k[:, :], start=True, stop=True)
                nc.vector.tensor_scalar_add(kcT[:, 128:M], pk3[:, 128:M, 0],
                                            biask[:, 0:1])
                nc.vector.tensor_add(kcT[:, 128:M], kcT[:, 128:M],
                                     pk3[:, 129 : M + 1, 1])
                nc.vector.memset(kcT[:, M:256], 0.0)
                # V compression
                pv = pvs.tile([128, 1024], F32, tag="pv", name="pvc")
                for c in range(32):
                    src = (vtA[:, 128 * c : 128 * (c + 1)] if c < 16 else
                           vtB[:, 128 * (c - 16) : 128 * (c - 15)])
                    nc.tensor.matmul(pv[:, 16 * c : 16 * c + 16], src,
                                     w01v[:, :], start=True, stop=True)
                pv3 = pv[:, 0:512].rearrange("p (t a) -> p t a", a=2)
                nc.vector.tensor_scalar_add(vcT[:, 0:M], pv3[:, 0:M, 0],
                                            biasv[:, 0:1])
                nc.vector.tensor_add(vcT[:, 0:M], vcT[:, 0:M],
                                     pv3[:, 1 : M + 1, 1])
                nc.vector.memset(vcT[:, M : M + 1], 0.0)
                tp = pvs.tile([128, 1024], F32, tag="pv", name="tp")
                nc.tensor.transpose(tp[:, 0:128], vcT[:, 0:128], ident[:, :])
                nc.tensor.transpose(tp[:, 128:256], vcT[:, 128:256],
                                    ident[:, :])
                nc.vector.tensor_copy(vca0[:, 0:128], tp[:, 0:128])
                nc.vector.tensor_copy(vca1[:, 0:128], tp[:, 128:256])

                def fetch_q(h):
                    qa = qp.tile([128, 2048], BF16, tag="qa", name="qa")
                    qb = qp.tile([128, 2048], BF16, tag="qb", name="qb")
                    nc.sync.dma_start(qa[:, :], qT_d.ap()[h, :, 0:2048])
                    nc.scalar.dma_start(qb[:, :], qT_d.ap()[h, :, 2048:4096])
                    return qa, qb

                qb_t[0] = q0b
                for h in range(4):
                    if h == 2:
                        nextq = fetch_q(4)
                    src = qb_t[h]   # [128, 2048] = blocks 4-7
                    for b in range(4, NBLK):
                        emit_block(h, b,
                                   src[:, 512 * (b - 4) : 512 * (b - 3)],
                                   oh_t[h])
                for h in range(4, HPC):
                    qa, qb = nextq
                    if h + 1 < HPC:
                        nextq = fetch_q(h + 1)
                    o_head = op.tile([128, N], BF16, tag="o", name="oh")
                    emit_pair(h, 0, qa[:, 0:512], qa[:, 512:1024], o_head)
                    emit_pair(h, 2, qa[:, 1024:1536], qa[:, 1536:2048],
                              o_head)
                    for b in range(4, NBLK):
                        emit_block(h, b,
                                   qb[:, 512 * (b - 4) : 512 * (b - 3)],
                                   o_head)
                        if h == HPC - 1:
                            # taper the pipeline so the post-loop tail is
                            # short
                            drain(max(2, TARGET - 2 * (b - 3)))
                drain(0)
    nc.compile()
    return nc


def make_consts(w_k, pe_k, w_v, pe_v):
    """Host-side constant tensors fed to every core."""
    f = np.float32
    w01k = np.zeros((128, 16), f)
    w01v = np.zeros((128, 16), f)
    for r in range(128):
        j = r // 16
        s = r % 16
        for a in range(2):
            # column layout (j, a): col = 2*j + a, matching psum (t, a)
            w01k[r, 2 * j + a] = w_k[16 * a + s]
            w01v[r, 2 * j + a] = w_v[16 * a + s]
    biask = (w_k[:, None] * pe_k).sum(0).astype(f)[:, None]  # [128,1]
    biasv = (w_v[:, None] * pe_v).sum(0).astype(f)[:, None]
    return {
        "w01k": np.ascontiguousarray(w01k).astype(ml_dtypes.bfloat16),
        "w01v": np.ascontiguousarray(w01v).astype(ml_dtypes.bfloat16),
        "biask": np.ascontiguousarray(biask),
        "biasv": np.ascontiguousarray(biasv),
    }


def make_in_map(q, k, v, consts, core):
    b, hq = core // 4, core % 4
    g = hq // 2
    qT = np.ascontiguousarray(
        q[b, :, 8 * hq : 8 * (hq + 1), :].transpose(1, 2, 0)
    ).astype(ml_dtypes.bfloat16)  # [8, D, N]
    # [N, D] -> SBUF tile layout [r=128, c=32, D] (r = row within chunk c)
    kk = np.ascontiguousarray(
        k[b, :, g, :].reshape(32, 128, D).transpose(1, 0, 2).reshape(128, 32 * D)
    ).astype(ml_dtypes.bfloat16)
    vv = np.ascontiguousarray(
        v[b, :, g, :].reshape(32, 128, D).transpose(1, 0, 2).reshape(128, 32 * D)
    ).astype(ml_dtypes.bfloat16)
    return {"qT": qT, "kk": kk, "vv": vv, **consts}


_CACHE = {}


def _compiled():
    if "nc" not in _CACHE:
        _CACHE["nc"] = build_program()
    return _CACHE["nc"]


def kernel(q, k, v, w_k, pe_k, w_v, pe_v, _trace=False, _trace_kwargs=None):
    q = np.asarray(q, np.float32)
    k = np.asarray(k, np.float32)
    v = np.asarray(v, np.float32)
    consts = make_consts(
        np.asarray(w_k, np.float32), np.asarray(pe_k, np.float32),
        np.asarray(w_v, np.float32), np.asarray(pe_v, np.float32),
    )
    nc = _compiled()
    in_maps = [make_in_map(q, k, v, consts, c) for c in range(8)]
    kw = {}
    if _trace:
        kw = {"trace": True, **(_trace_kwargs or {})}
    res = run_bass_kernel_spmd(nc, in_maps, core_ids=list(range(8)), **kw)
    out = np.empty((B, N, QH, VD), np.float32)
    for c in range(8):
        b, hq = c // 4, c % 4
        # o: [HPC, 128 p, 32 t, VD]; query n = 128*t + p
        oc = np.asarray(res.results[c]["o"], dtype=np.float32)
        out[b, :, 8 * hq : 8 * (hq + 1), :] = (
            oc.transpose(2, 1, 0, 3).reshape(N, HPC, VD)
        )
    out[:, : KSZ - 1] = 0.0   # queries with no visible block (device NaNs)
    _CACHE["last_result"] = res
    return out
